# revision 6
# baseline (speedup 1.0000x reference)
"""ChannelBlockImportanceGate kernel for 8 Trainium2 NeuronCores.

Computes, per (b, c) slice of features [8, 256, 132, 132] f32:
  scores = block-sum of |x| over 8x8 blocks (17x17 grid, zero-padded edges)
  top-72 blocks (ties -> lowest index, matching jax.lax.top_k)
  output = per-pixel {0,1} mask upsampled 8x8 (cropped to 132x132)

The straight-through soft term of the reference cancels in the forward
pass (hard - sg(soft) + soft == hard up to ~1ulp), so the output is the
hard mask.

Sharding: purely data parallel. 2048 (b,c) slices -> 256 per core.
Per core: 2 groups of 128 slices; each slice occupies one SBUF
partition so pooling/topk/upsample are per-partition ops with no
cross-partition traffic. Top-72 uses 9 rounds of DVE max8 +
match_replace(-1e30), then mask = (score < 0).

v2 design (from perfetto analysis of v1):
 - The 16 SDMA engines each need ~83us busy (load f32 + store f32
   sides both run at the ~27GB/s per-engine port rate; cast-stores
   do NOT reduce engine busy time -- measured). v1's DMA_15 ran ~24%
   slower than its peers (101.5us busy, zero idle -> it WAS the
   critical path). The suspected cause is GpSimd Q7 SBUF-port
   contention from the 3.7us gpsimd tensor_copies; v2 has NO gpsimd
   compute at all.
 - The mask is materialized PACKED: one f32 element = 4 fp8(0|1)
   pixels (mask * 0x38383838-as-f32, exact since mask is {0,1}).
   Upsample copies move 4x fewer elements (4.8us/group on one
   engine instead of 19us), done by scalar+vector, freeing gpsimd.
 - Stores are SWDGE (nc.gpsimd.dma_start) with fp8->f32 cast into
   the f32 output (bit-exact, verified). They ride the Pool queue,
   separate from the sync-queue loads, so engines round-robin
   load/store packets instead of FIFOing through one queue.
 - Vector chain: pool g0 (18.7us) -> topk g0 (9.5us) -> pool g1 ->
   topk g1, finishing ~70us so the g1 store stream (last ~21us of
   per-engine DMA work) is descriptor-fed in time.
"""

import numpy as np

B, C, H, W = 8, 256, 132, 132
HW = H * W            # 17424
NB = 17               # 8x8 blocks per side (132 padded to 136)
NBLK = NB * NB        # 289
KEEP = 72             # round(289 * 0.25)
N_CORES = 8
S = (B * C) // N_CORES  # 256 slices per core
WP = W // 4           # 33 packed u32 per pixel row
LOAD_CHUNKS = ((0, 8), (8, 40), (40, 72), (72, 104), (104, 132))
STORE_CHUNKS = ((0, 32), (32, 64), (64, 96), (96, 132))
NEG = -1.0e30
# f32 whose 4 bytes are each fp8e4m3(1.0) = 0x38; mask * PACK4 produces
# the packed 4-pixel fp8 row exactly (mask is exactly 0.0 or 1.0).
PACK4 = float(np.frombuffer(np.uint32(0x38383838).tobytes(),
                            dtype=np.float32)[0])

_prog_cache = {}


def _build_program():
    import concourse.bacc as bacc
    import concourse.mybir as mybir
    import concourse.tile as tile

    f32 = mybir.dt.float32
    fp8 = mybir.dt.float8e4
    X = mybir.AxisListType.X
    XY = mybir.AxisListType.XY
    ADD = mybir.AluOpType.add

    nc = bacc.Bacc("TRN2", debug=False, num_devices=N_CORES)
    x = nc.dram_tensor("x", (S, HW), f32, kind="ExternalInput")
    y = nc.dram_tensor("y", (S, HW), f32, kind="ExternalOutput")

    with tile.TileContext(nc) as tc:
        with (
            tc.tile_pool(name="big", bufs=2) as bigp,
            tc.tile_pool(name="med", bufs=2) as medp,
            tc.tile_pool(name="small", bufs=2) as smallp,
        ):
            # All load DMAs are emitted before any store DMA. Loads ride
            # the sync HWDGE queue (g0's first chunk on the scalar HWDGE
            # queue, measured fastest in v1); stores ride the Pool
            # (SWDGE) queue so load/store packets interleave round-robin
            # on each SDMA engine.
            xb = []
            li = 0
            for g in range(S // 128):
                p0 = g * 128
                xt = bigp.tile([128, HW], f32, name=f"xb_g{g}", tag="xb")
                for k, (r0, r1) in enumerate(LOAD_CHUNKS):
                    # Alternate the two HWDGE queues: per-DMA completion
                    # overhead serializes within a queue but overlaps
                    # across queues (mb2 vs mb4: 117us -> 97us for the
                    # same 24 chunked DMAs).
                    eng = nc.sync if li % 2 == 0 else nc.scalar
                    li += 1
                    eng.dma_start(out=xt[:, r0 * W:r1 * W],
                                  in_=x[p0:p0 + 128, r0 * W:r1 * W])
                xb.append(xt)

            for g in range(S // 128):
                p0 = g * 128
                xt = xb[g]
                xv = xt.rearrange("p (r w) -> p r w", w=W)

                # Fused 8x8 block pooling: one XY tensor_reduce per chunk
                # computes scores[p, h, q] = sum |x| over the full 8x8
                # block directly from the raw pixels. Edge strips are 3
                # small XY reduces.
                scores = smallp.tile([128, NBLK], f32,
                                     name=f"scores_g{g}", tag="scores")
                sc3 = scores.rearrange("p (h t) -> p h t", t=NB)
                if g > 0:
                    # Ordering token (from v1): pins this group's pooling
                    # after the previous group's mask on the vector
                    # engine, otherwise the scheduler interleaves the two
                    # groups' pooling and delays the first mask by ~20us.
                    nc.vector.tensor_copy(
                        out=scores[0:1, :],
                        in_=prev_pm[0:1, 0:1].broadcast_to((1, NBLK)))
                for k, (r0, r1) in enumerate(LOAD_CHUNKS):
                    rr1 = min(r1, 128)
                    nc.vector.tensor_reduce(
                        out=sc3[:, r0 // 8:rr1 // 8, 0:16],
                        in_=(xv[:, r0:rr1, 0:128]
                             .rearrange("p (h r) (q c) -> p h q r c",
                                        r=8, c=8)),
                        axis=XY, op=ADD, apply_absolute_value=True)
                nc.vector.tensor_reduce(
                    out=sc3[:, 0:16, 16:17],
                    in_=(xv[:, 0:128, 128:132]
                         .rearrange("p (h r) c -> p h r c", r=8)),
                    axis=XY, op=ADD, apply_absolute_value=True)
                nc.vector.tensor_reduce(
                    out=sc3[:, 16:17, 0:16],
                    in_=(xv[:, 128:132, 0:128]
                         .rearrange("p r (q c) -> p q r c", c=8)),
                    axis=XY, op=ADD, apply_absolute_value=True)
                nc.vector.tensor_reduce(
                    out=sc3[:, 16:17, 16:17],
                    in_=xv[:, 128:132, 128:132].unsqueeze(1),
                    axis=XY, op=ADD, apply_absolute_value=True)

                # Top-72 per partition: 9 rounds of max8 + match_replace.
                # match_replace replaces the first unmatched occurrence, so
                # ties resolve to the lowest index like jax.lax.top_k.
                for it in range(KEEP // 8):
                    m8 = smallp.tile([128, 8], f32,
                                     name=f"m8_g{g}i{it}", tag="m8")
                    nc.vector.max(out=m8[:, :], in_=scores[:, :])
                    nc.vector.match_replace(out=scores[:, :],
                                            in_to_replace=m8[:, :],
                                            in_values=scores[:, :],
                                            imm_value=NEG)

                # Packed block mask: replaced entries are -1e30, so
                # (score < 0) * PACK4 writes 0x38383838 (4 fp8 ones) for
                # selected blocks, 0.0 for the rest. pm[p, h*17+q].
                pm = smallp.tile([128, NBLK], f32, name=f"pm_g{g}",
                                 tag="pm")
                nc.vector.tensor_scalar(out=pm[:, :], in0=scores[:, :],
                                        scalar1=0.0, scalar2=PACK4,
                                        op0=mybir.AluOpType.is_lt,
                                        op1=mybir.AluOpType.mult)
                pm3 = pm.rearrange("p (h t) -> p h t", t=NB)
                prev_pm = pm

                # Packed row-mask [p, h, 33]: one 132-px row (33 packed
                # elems) per block-row; blocks 0..15 span 2 packed elems
                # each, edge block 16 spans exactly 1 (pixels 128-131).
                rm = medp.tile([128, NB * WP], f32, name=f"rm_g{g}",
                               tag="rm")
                rm3 = rm.rearrange("p (h w) -> p h w", w=WP)
                nc.vector.tensor_copy(
                    out=rm3[:, :, 0:32].rearrange("p h (q c) -> p h q c",
                                                  c=2),
                    in_=(pm3[:, :, 0:16].unsqueeze(3)
                         .broadcast_to((128, NB, 16, 2))))
                nc.vector.tensor_copy(
                    out=rm3[:, :, 32:33],
                    in_=pm3[:, :, 16:17])

                # Vertical 8x upsample into the packed mask tile, then
                # SWDGE cast-store (fp8 -> f32) per chunk. Upsample is 4x
                # fewer elements than v1 (packed), split scalar/vector.
                mk = medp.tile([128, NB * 8 * WP], f32, name=f"mk_g{g}",
                               tag="mk")
                mk4 = mk.rearrange("p (h r w) -> p h r w", r=8, w=WP)
                for k, (r0, r1) in enumerate(STORE_CHUNKS):
                    h0, h1 = r0 // 8, (r1 + 7) // 8
                    nr = min(r1, 136) - r0
                    src = (rm3[:, h0:h1, :].unsqueeze(2)
                           .broadcast_to((128, h1 - h0, 8, WP)))
                    dst = mk4[:, h0:h1, :, :]
                    nc.scalar.copy(out=dst, in_=src)
                    # Store rows r0:r1 (crop block-row 16 to 4 rows via
                    # the flat view of mk).
                    nc.gpsimd.dma_start(
                        out=y[p0:p0 + 128, r0 * W:r1 * W],
                        in_=mk[:, r0 * WP:r1 * WP].bitcast(fp8))
    nc.compile()
    return nc


def _ensure_ntff_hook_module():
    """bass_utils' trace path does `from antenv.axon_hooks import
    get_axon_ntff_profile_hook` — a module this image doesn't ship.
    Register an equivalent (ctypes into libaxon_pjrt.so, mirroring
    trn_boot._ntff_profile_via_ctypes) so BASS_TRACE=1 works; degrade
    to a None hook (trace skipped) when unavailable."""
    import sys
    import types

    try:
        import antenv.axon_hooks  # noqa: F401
        return
    except Exception:
        pass

    hook = None
    try:
        import contextlib
        import ctypes

        so_path = "/opt/axon/libaxon_pjrt.so"
        lib = ctypes.CDLL(so_path)
        if hasattr(lib, "axon_start_nrt_profile"):
            lib.axon_start_nrt_profile.argtypes = [
                ctypes.POINTER(ctypes.c_int64), ctypes.c_size_t]
            lib.axon_start_nrt_profile.restype = ctypes.c_int64
            lib.axon_stop_nrt_profile.argtypes = [ctypes.c_char_p]
            lib.axon_stop_nrt_profile.restype = ctypes.c_int64

            @contextlib.contextmanager
            def _hook(output_dir, device_ids):
                import jax
                jax.devices()
                if device_ids:
                    ids = (ctypes.c_int64 * len(device_ids))(*device_ids)
                    rc = lib.axon_start_nrt_profile(ids, len(device_ids))
                else:
                    rc = lib.axon_start_nrt_profile(None, 0)
                if rc != 0:
                    raise RuntimeError(f"axon_start_nrt_profile rc={rc}")
                try:
                    yield
                finally:
                    n = lib.axon_stop_nrt_profile(str(output_dir).encode())
                    print(f"ntff profile: {n} file(s) -> {output_dir}",
                          file=sys.stderr)

            hook = _hook
    except Exception:
        hook = None

    mod = types.ModuleType("antenv.axon_hooks")
    mod.get_axon_ntff_profile_hook = lambda: hook
    mod.set_axon_ntff_profile_hook = lambda h: None
    sys.modules["antenv.axon_hooks"] = mod


def _get_program():
    if "nc" not in _prog_cache:
        _prog_cache["nc"] = _build_program()
    return _prog_cache["nc"]


def kernel(features, enabled):
    feats = np.asarray(features)
    if not bool(np.asarray(enabled)):
        return np.ones(feats.shape, dtype=np.float32)

    _ensure_ntff_hook_module()
    import concourse.bass_utils as _bu
    from concourse.bass_utils import run_bass_kernel_spmd

    # The trace path uploads artifacts to a shared bucket; tolerate
    # sandboxes where that fails.
    if not getattr(_bu, "_upload_patched", False):
        _orig_upload = _bu.upload_artifacts

        def _safe_upload(tmpdir):
            try:
                return _orig_upload(tmpdir)
            except Exception:
                return str(tmpdir)

        _bu.upload_artifacts = _safe_upload
        _bu._upload_patched = True

    nc = _get_program()
    flat = np.ascontiguousarray(feats.reshape(B * C, HW), dtype=np.float32)
    in_maps = [{"x": flat[i * S:(i + 1) * S]} for i in range(N_CORES)]
    res = run_bass_kernel_spmd(nc, in_maps, list(range(N_CORES)))
    _prog_cache["last_res"] = res
    out = np.concatenate([np.asarray(res.results[i]["y"])
                          for i in range(N_CORES)], axis=0)
    return out.reshape(B, C, H, W).astype(np.float32)
